# revision 52
# baseline (speedup 1.0000x reference)
"""Barycentric-coordinates KNN kernel for Trainium2 (8 NeuronCores).

Pipeline (per core = one (batch, half-of-V) pair; 8 cores cover 4 batches x 2 halves):
  Phase 1 (device): -d^2 via a single 16-row fp16 split-precision TensorE
    matmul (2q.p - |p|^2 - |q|^2 accumulated in fp32 PSUM, ~1e-6 abs error);
    a 7-bit chunk-local index packed into the mantissa low bits so one DVE
    MAX8 per 128-wide chunk yields fused (value, index) -> 256 candidates/row.
  Host: decode, exact f32 re-score of the top-48 candidates (erases fp16 +
    mask noise), top-33 by (d2, index), neighbor gather, SHOT weights (no
    per-partition gather exists on-chip).
  Phase 2 (device): weighted 3x3 covariance, eigensolver (trig closed-form
    roots of the characteristic cubic via ACT Arctan/Sin + 2 Newton polish
    steps, adjugate cross-products; the two eigenvector chains run
    concurrently on DVE and GpSimd), SHOT sign disambiguation, tangent-plane
    log map -> pxy out; template-cell nearest-3 selection with the polar
    expansion key (S2[k] + r_i^2) - 2 r_i (cos_j px + sin_j py), bit-packed
    (d^2 mantissa | k-slot), MAX8 per cell -> m3 keys out.
  Host: decode k-slots, gather winner coords from pxy, barycentric weights,
    pidx = nbr_idx[closest], assemble (4, 4096, 5, 8, 3, 2) output.
Device outputs are validated (plausible d^2 range, |p|^2 vs shipped
distances, distinct slots) with up to 2 retries to guard against rare
transient hardware flakes.
"""
import sys

sys.path.insert(0, "/opt/trn_rl_repo")

import numpy as np
from contextlib import ExitStack

import concourse.bass as bass
import concourse.mybir as mybir
import concourse.tile as tile
from concourse.bass_utils import run_bass_kernel_spmd
from concourse.tile import ScopedClock

f32 = np.float32
AF = mybir.ActivationFunctionType
ALU = mybir.AluOpType
DT = mybir.dt

B, V, K = 4, 4096, 32
HALF = V // 2            # queries per core
NT = HALF // 128         # 16 v-tiles per core
NCHUNK = 32              # phase-1 chunk count (chunk width 128)
CHUNKW = V // NCHUNK     # 128
CAND = NCHUNK * 8        # 256 candidates per row
R, A = 5, 8
NCELL = R * A            # 40 template cells
EPS = 1e-8
N_RADIAL, N_ANGULAR = 5, 8
TEMPLATE_RADIUS = 0.09

# ---------------------------------------------------------------------------
# Tile-framework workaround: walrus rejects instructions carrying more than a
# couple of sync waits. Spread extras across single-wait NOPs.
# ---------------------------------------------------------------------------


def _patched_drain_and_barrier(self, tick_clock, wait_clock):
    probe = self.nc.sync.nop(nofuse=True)
    wait_clock.add_sem_waits(probe.ins, ScopedClock({None: tick_clock.global_clock}))
    sync_info = probe.ins.sync_info
    waits = list(sync_info.on_wait or []) if sync_info is not None else []
    if len(waits) > 1:
        sync_info.on_wait = waits[:1]
        for i in range(1, len(waits)):
            extra = self.nc.sync.nop(nofuse=True)
            if extra.ins.sync_info is None:
                extra.ins.sync_info = mybir.SyncInfo(on_wait=[waits[i]], on_update=[])
            else:
                extra.ins.sync_info.on_wait = [waits[i]]
    self.nc.sync.drain()
    self.nc.all_engine_barrier()
    assert self.sems is not None
    popped = self.nc._tile_sem_poison_stack.pop()
    assert popped is self._sem_poison
    self.nc.clear_and_free_semaphores(list(self.sems.allocated().values()))
    self.nc.all_engine_barrier()


tile.TileContext._drain_and_barrier = _patched_drain_and_barrier


def split_sync_waits(nc, max_waits=1):
    for f in nc.m.functions:
        for b in f.blocks:
            new_list = []
            dirty = False
            for ins in b.instructions:
                si = ins.sync_info
                waits = list(si.on_wait) if (si is not None and si.on_wait) else []
                if len(waits) > max_waits:
                    dirty = True
                    extras, keep = waits[:-max_waits], waits[-max_waits:]
                    for j in range(0, len(extras), max_waits):
                        nop = mybir.InstNoOp(
                            name=f"I-wsplit-{nc.next_id()}", engine=ins.engine
                        )
                        nop.sync_info = mybir.SyncInfo(
                            on_wait=extras[j : j + max_waits], on_update=[]
                        )
                        new_list.append(nop)
                    si.on_wait = keep
                new_list.append(ins)
            if dirty:
                b.instructions = new_list


# ---------------------------------------------------------------------------
# Phase 1 program
# ---------------------------------------------------------------------------


def build_phase1():
    # -d2(q, p) via one 13-row fp16 split-precision matmul:
    #   2 q.p  = sum_c (ah_c + al_c)(bh_c + bl_c)  (al.bl term dropped)
    #   -|p|^2 = sph + spl,  -|q|^2 = sqh + sql    (hi/lo fp16 splits)
    # accumulated exactly in fp32 PSUM -> -d^2 with ~1e-6 abs error.
    # A 7-bit chunk-local index is packed into the mantissa low bits so a
    # single MAX8 per 128-wide chunk yields (value, index) fused; the host
    # decodes idx = bits & 127.
    nc = bass.Bass()
    NROW = 16
    pt5 = nc.declare_dram_parameter("pt5", [NROW, V], DT.float16, isOutput=False)
    qt5 = nc.declare_dram_parameter("qt5", [NROW, HALF], DT.float16, isOutput=False)
    candv_o = nc.declare_dram_parameter("candv", [HALF, CAND], DT.float32, isOutput=True)

    HC = NCHUNK // 2  # chunks per half (16)

    with tile.TileContext(nc) as tc, ExitStack() as ctx:
        cpool = ctx.enter_context(tc.tile_pool(name="const", bufs=1))
        npool = ctx.enter_context(tc.tile_pool(name="nkey", bufs=3))
        opool = ctx.enter_context(tc.tile_pool(name="cand", bufs=4))
        ppool = ctx.enter_context(tc.tile_pool(name="psum", bufs=2, space="PSUM"))

        pt = cpool.tile([NROW, V], DT.float16)
        qt = cpool.tile([NROW, HALF], DT.float16)
        J7 = cpool.tile([128, 2048], DT.int32)
        M128 = cpool.tile([128, 1], DT.int32)
        nc.sync.dma_start(pt[:], pt5[:])
        nc.sync.dma_start(qt[:], qt5[:])
        nc.gpsimd.iota(J7[:], pattern=[[0, HC], [1, CHUNKW]], base=0,
                       channel_multiplier=0)
        nc.vector.memset(M128[:], -128)

        for t in range(NT):
            for jh in range(2):
                ps = ppool.tile([128, 2048], DT.float32, space="PSUM")
                for k4 in range(4):
                    nc.tensor.matmul(
                        ps[:, k4 * 512:(k4 + 1) * 512],
                        qt[:, t * 128:(t + 1) * 128],
                        pt[:, jh * 2048 + k4 * 512: jh * 2048 + (k4 + 1) * 512],
                        start=True, stop=True,
                    )
                nk = npool.tile([128, 2048], DT.float32, tag="nk")
                nc.vector.scalar_tensor_tensor(
                    out=nk[:].bitcast(DT.int32), in0=ps[:].bitcast(DT.int32),
                    scalar=M128[:], in1=J7[:], op0=ALU.bitwise_and,
                    op1=ALU.bitwise_or)
                cv = opool.tile([128, HC * 8], DT.float32, tag="cv")
                for c in range(HC):
                    nc.vector.max(out=cv[:, c * 8:(c + 1) * 8],
                                  in_=nk[:, c * CHUNKW:(c + 1) * CHUNKW])
                nc.sync.dma_start(
                    candv_o[t * 128:(t + 1) * 128, jh * HC * 8:(jh + 1) * HC * 8],
                    cv[:])

    split_sync_waits(nc)
    return nc


# ---------------------------------------------------------------------------
# Phase 2 program
# ---------------------------------------------------------------------------


def _register_consts(nc, values):
    for value in values:
        t = nc.alloc_sbuf_tensor(f"const-float32-{value}", [128, 1], DT.float32)
        nc.gpsimd.memset(t.ap(), value)
        nc.const_aps.aps[(DT.float32, value)] = t.ap()
    nc.all_engine_barrier()


def build_phase2():
    nc = bass.Bass()
    _register_consts(nc, [0.5, float(np.pi / 2), float(-np.pi / 6)])
    ngh_i = nc.declare_dram_parameter("ngh", [HALF, 96], DT.float32, isOutput=False)
    wn_i = nc.declare_dram_parameter("wn", [HALF, K], DT.float32, isOutput=False)
    dd_i = nc.declare_dram_parameter("dd", [HALF, K], DT.float32, isOutput=False)
    # cst layout: [0:8]=-2cos(a), [8:16]=-2sin(a), [16:21]=r, [21:26]=r^2,
    #             [26:66]=tx, [66:106]=ty   (replicated over partitions)
    cst_i = nc.declare_dram_parameter("cst", [128, 106], DT.float32, isOutput=False)
    m3_o = nc.declare_dram_parameter("m3o", [HALF, NCELL, 3], DT.float32, isOutput=True)
    pxy_o = nc.declare_dram_parameter("pxy", [HALF, 2, K], DT.float32, isOutput=True)

    with tile.TileContext(nc) as tc, ExitStack() as ctx:
        cp = ctx.enter_context(tc.tile_pool(name="const", bufs=1))
        sp = ctx.enter_context(tc.tile_pool(name="scratch", bufs=2))
        bp = ctx.enter_context(tc.tile_pool(name="bc", bufs=2))

        NGH = cp.tile([128, NT, 96], DT.float32)
        WN = cp.tile([128, NT, K], DT.float32)
        DD = cp.tile([128, NT, K], DT.float32)
        CST = cp.tile([128, 106], DT.float32)
        HQ = HALF // 2
        # ngh/wn land first (in halves) so covariance starts before the
        # rest of the input DMA completes
        for th in range(2):
            rs = slice(th * HQ, (th + 1) * HQ)
            ts = slice(th * (NT // 2), (th + 1) * (NT // 2))
            nc.sync.dma_start(NGH[:, ts],
                              ngh_i[rs].rearrange("(t p) c -> p t c", p=128))
            nc.sync.dma_start(WN[:, ts],
                              wn_i[rs].rearrange("(t p) c -> p t c", p=128))
        nc.sync.dma_start(DD[:], dd_i[:].rearrange("(t p) c -> p t c", p=128))
        nc.sync.dma_start(CST[:], cst_i[:])
        TX = CST[:, 26:66]
        TY = CST[:, 66:106]

        KIOTA = cp.tile([128, NCELL, K], DT.int32)
        nc.gpsimd.iota(KIOTA[:], pattern=[[0, NCELL], [1, K]], base=-2147483648,
                       channel_multiplier=0)
        M32 = cp.tile([128, 1], DT.int32)
        nc.vector.memset(M32[:], -32)

        _tagn = [0]

        def nt_tile(pool=cp):
            _tagn[0] += 1
            return pool.tile([128, NT], DT.float32, tag=f"nt{_tagn[0]}",
                             name=f"nt{_tagn[0]}")

        NGH4 = NGH[:].rearrange("p t (c k) -> p t c k", c=3)

        # ---- covariance accumulation (batched per tile-half) ----
        NW = cp.tile([128, NT, 96], DT.float32)
        NW4 = NW[:].rearrange("p t (c k) -> p t c k", c=3)
        CXX, CXY, CXZ, CYY, CYZ, CZZ = [nt_tile() for _ in range(6)]
        cov_dsts = {"xx": CXX, "xy": CXY, "xz": CXZ, "yy": CYY, "yz": CYZ, "zz": CZZ}
        pairs = [("xx", 0, 0), ("xy", 0, 1), ("xz", 0, 2),
                 ("yy", 1, 1), ("yz", 1, 2), ("zz", 2, 2)]
        for th in range(2):
            ts = slice(th * (NT // 2), (th + 1) * (NT // 2))
            HTC = NT // 2
            nc.gpsimd.tensor_tensor(
                out=NW4[:, ts], in0=NGH4[:, ts],
                in1=WN[:, ts].rearrange("p t k -> p t () k").to_broadcast(
                    [128, HTC, 3, K]),
                op=ALU.mult)
            for nmq, a, b in pairs:
                cj = sp.tile([128, HTC, K], DT.float32, tag="covjunk")
                nc.gpsimd.tensor_tensor(out=cj[:], in0=NGH4[:, ts, a, :],
                                        in1=NW4[:, ts, b, :], op=ALU.mult)
                nc.vector.tensor_reduce(out=cov_dsts[nmq][:, ts], in_=cj[:],
                                        axis=mybir.AxisListType.X, op=ALU.add)

        # ---- eigensolver on (128, NT) ----
        def tt(dst, a, bb, op):
            nc.vector.tensor_tensor(out=dst[:], in0=a[:], in1=bb[:], op=op)

        def sq_act(dst, a):
            nc.vector.tensor_tensor(out=dst[:], in0=a[:], in1=a[:], op=ALU.mult)

        Q = nt_tile()
        tt(Q, CXX, CYY, ALU.add)
        tt(Q, Q, CZZ, ALU.add)
        nc.vector.tensor_scalar_mul(Q[:], Q[:], 1.0 / 3.0)
        BXX, BYY, BZZ = nt_tile(), nt_tile(), nt_tile()
        tt(BXX, CXX, Q, ALU.subtract)
        tt(BYY, CYY, Q, ALU.subtract)
        tt(BZZ, CZZ, Q, ALU.subtract)
        P2 = nt_tile()
        T1 = nt_tile(sp)
        sq_act(P2, BXX)
        sq_act(T1, BYY)
        tt(P2, P2, T1, ALU.add)
        sq_act(T1, BZZ)
        tt(P2, P2, T1, ALU.add)
        T2 = nt_tile(sp)
        sq_act(T1, CXY)
        sq_act(T2, CXZ)
        tt(T1, T1, T2, ALU.add)
        sq_act(T2, CYZ)
        tt(T1, T1, T2, ALU.add)
        nc.vector.tensor_scalar_mul(T1[:], T1[:], 2.0)
        tt(P2, P2, T1, ALU.add)
        PP = nt_tile()
        PPX = nt_tile()
        nc.vector.tensor_scalar_mul(PPX[:], P2[:], 1.0 / 6.0)

        def polished_sqrt(dst, x, tmp):
            # ACT Sqrt is ~7e-6; one Newton step s' = (s + x/s)/2 fixes it
            nc.scalar.activation(dst[:], x[:], AF.Sqrt)
            nc.vector.tensor_scalar_max(tmp[:], dst[:], 1e-30)
            nc.vector.reciprocal(tmp[:], tmp[:])
            nc.vector.tensor_tensor(out=tmp[:], in0=x[:], in1=tmp[:], op=ALU.mult)
            nc.vector.tensor_tensor(out=dst[:], in0=dst[:], in1=tmp[:], op=ALU.add)
            nc.vector.tensor_scalar_mul(dst[:], dst[:], 0.5)

        polished_sqrt(PP, PPX, T2)
        PINV = nt_tile()
        nc.vector.tensor_scalar_max(PINV[:], PP[:], 1e-20)
        nc.vector.reciprocal(PINV[:], PINV[:])
        NBXX, NBYY, NBZZ, NBXY, NBXZ, NBYZ = [nt_tile() for _ in range(6)]
        tt(NBXX, BXX, PINV, ALU.mult)
        tt(NBYY, BYY, PINV, ALU.mult)
        tt(NBZZ, BZZ, PINV, ALU.mult)
        tt(NBXY, CXY, PINV, ALU.mult)
        tt(NBXZ, CXZ, PINV, ALU.mult)
        tt(NBYZ, CYZ, PINV, ALU.mult)
        # det(B̂)
        DET = nt_tile()
        sq_act(T1, NBYZ)                     # byz^2
        tt(T2, NBYY, NBZZ, ALU.mult)
        tt(T2, T2, T1, ALU.subtract)
        tt(DET, NBXX, T2, ALU.mult)          # + bxx (byy bzz - byz^2)
        tt(T1, NBXY, NBZZ, ALU.mult)
        tt(T2, NBYZ, NBXZ, ALU.mult)
        tt(T1, T1, T2, ALU.subtract)
        tt(T1, NBXY, T1, ALU.mult)
        tt(DET, DET, T1, ALU.subtract)       # - bxy (bxy bzz - byz bxz)
        tt(T1, NBXY, NBYZ, ALU.mult)
        tt(T2, NBYY, NBXZ, ALU.mult)
        tt(T1, T1, T2, ALU.subtract)
        tt(T1, NBXZ, T1, ALU.mult)
        tt(DET, DET, T1, ALU.add)            # + bxz (bxy byz - byy bxz)
        R2 = nt_tile()                       # 2r = det  clamped to [-2, 2]
        nc.vector.tensor_scalar_min(R2[:], DET[:], 2.0)
        nc.vector.tensor_scalar_max(R2[:], R2[:], -2.0)

        # Roots of beta^3 - 3 beta = 2c via beta = 2 cos(acos(c)/3 - 2 pi k/3):
        # trig seed from ACT Arctan/Sin tables, then 2 Newton polish steps.
        CC = nt_tile()
        nc.vector.tensor_scalar_mul(CC[:], R2[:], 0.5)          # c in [-1, 1]
        OM = nt_tile(sp)
        sq_act(OM, CC)
        nc.vector.tensor_scalar(out=OM[:], in0=OM[:], scalar1=-1.0, scalar2=1.0,
                                op0=ALU.mult, op1=ALU.add)      # 1 - c^2
        nc.vector.tensor_scalar_max(OM[:], OM[:], 1e-12)
        SRT = nt_tile(sp)
        nc.scalar.activation(SRT[:], OM[:], AF.Sqrt)
        nc.vector.tensor_scalar_max(SRT[:], SRT[:], 1e-10)
        nc.vector.reciprocal(SRT[:], SRT[:])
        AT = nt_tile(sp)
        tt(AT, CC, SRT, ALU.mult)                               # tan(arcsin c)
        nc.vector.tensor_scalar_min(AT[:], AT[:], 50.0)
        nc.vector.tensor_scalar_max(AT[:], AT[:], -50.0)
        nc.scalar.activation(AT[:], AT[:], AF.Arctan)           # arcsin(c)
        PHI3 = nt_tile(sp)
        nc.vector.tensor_scalar(out=PHI3[:], in0=AT[:], scalar1=-1.0 / 3.0,
                                scalar2=float(np.pi / 6), op0=ALU.mult,
                                op1=ALU.add)                    # acos(c)/3
        BMAX = nt_tile()
        BMIN = nt_tile()
        # beta_max = 2 cos(phi/3) = 2 sin(pi/2 - phi/3)
        nc.scalar.activation(BMAX[:], PHI3[:], AF.Sin, bias=float(np.pi / 2),
                             scale=-1.0)
        nc.vector.tensor_scalar_mul(BMAX[:], BMAX[:], 2.0)
        # beta_min = 2 cos(phi/3 + 2pi/3) = 2 sin(-pi/6 - phi/3)
        nc.scalar.activation(BMIN[:], PHI3[:], AF.Sin, bias=float(-np.pi / 6),
                             scale=-1.0)
        nc.vector.tensor_scalar_mul(BMIN[:], BMIN[:], 2.0)

        def polish(BETA, E, iters=2):
            FV = nt_tile(sp)
            B2 = nt_tile(sp)
            TP = nt_tile(sp)
            for _ in range(iters):
                E.tensor_tensor(out=B2[:], in0=BETA[:], in1=BETA[:], op=ALU.mult)
                E.tensor_tensor(out=FV[:], in0=B2[:], in1=BETA[:], op=ALU.mult)
                E.tensor_scalar_mul(TP[:], BETA[:], 3.0)
                E.tensor_tensor(out=TP[:], in0=TP[:], in1=FV[:], op=ALU.subtract)
                E.tensor_tensor(out=TP[:], in0=TP[:], in1=R2[:], op=ALU.add)  # -f
                E.tensor_scalar(out=B2[:], in0=B2[:], scalar1=3.0,
                                scalar2=-3.0, op0=ALU.mult, op1=ALU.add)
                E.tensor_scalar_max(B2[:], B2[:], 1e-8)
                nc.vector.reciprocal(B2[:], B2[:])
                E.tensor_tensor(out=TP[:], in0=TP[:], in1=B2[:], op=ALU.mult)
                E.tensor_tensor(out=BETA[:], in0=BETA[:], in1=TP[:], op=ALU.add)

        polish(BMAX, nc.vector)
        polish(BMIN, nc.vector)
        LMAX = nt_tile()
        LMIN = nt_tile()
        tt(LMAX, PP, BMAX, ALU.mult)
        tt(LMAX, LMAX, Q, ALU.add)
        tt(LMIN, PP, BMIN, ALU.mult)
        tt(LMIN, LMIN, Q, ALU.add)

        def evec(lam, E):
            # columns of A - lam I. E = engine for the bulk TT work; is_ge
            # picks and reciprocal stay on DVE (Pool lacks them).
            TM = nt_tile(sp)

            def tte(dst, a, bb, op):
                E.tensor_tensor(out=dst[:], in0=a[:], in1=bb[:], op=op)

            D0, D1, D2 = nt_tile(sp), nt_tile(sp), nt_tile(sp)
            tte(D0, CXX, lam, ALU.subtract)
            tte(D1, CYY, lam, ALU.subtract)
            tte(D2, CZZ, lam, ALU.subtract)
            m0 = (D0, CXY, CXZ)
            m1 = (CXY, D1, CYZ)
            m2 = (CXZ, CYZ, D2)

            def cross(u, v):
                rx, ry, rz = nt_tile(sp), nt_tile(sp), nt_tile(sp)
                tte(rx, u[1], v[2], ALU.mult)
                tte(TM, u[2], v[1], ALU.mult)
                tte(rx, rx, TM, ALU.subtract)
                tte(ry, u[2], v[0], ALU.mult)
                tte(TM, u[0], v[2], ALU.mult)
                tte(ry, ry, TM, ALU.subtract)
                tte(rz, u[0], v[1], ALU.mult)
                tte(TM, u[1], v[0], ALU.mult)
                tte(rz, rz, TM, ALU.subtract)
                return rx, ry, rz

            def norm2(c):
                n = nt_tile(sp)
                tte(n, c[0], c[0], ALU.mult)
                tte(TM, c[1], c[1], ALU.mult)
                tte(n, n, TM, ALU.add)
                tte(TM, c[2], c[2], ALU.mult)
                tte(n, n, TM, ALU.add)
                return n

            c01 = cross(m0, m1)
            c02 = cross(m0, m2)
            c12 = cross(m1, m2)
            n01, n02, n12 = norm2(c01), norm2(c02), norm2(c12)
            G1, G2, G3 = nt_tile(sp), nt_tile(sp), nt_tile(sp)
            tt(G1, n01, n02, ALU.is_ge)
            tt(G2, n01, n12, ALU.is_ge)
            tte(G1, G1, G2, ALU.mult)                   # pick01
            tt(G3, n02, n12, ALU.is_ge)
            U = nt_tile(sp)
            E.tensor_scalar(out=U[:], in0=G1[:], scalar1=-1.0, scalar2=1.0,
                            op0=ALU.mult, op1=ALU.add)   # 1 - pick01
            tte(G2, U, G3, ALU.mult)                    # pick02
            E.tensor_scalar(out=G3[:], in0=G3[:], scalar1=-1.0, scalar2=1.0,
                            op0=ALU.mult, op1=ALU.add)   # 1 - g3
            tte(G3, U, G3, ALU.mult)                    # pick12
            out = []
            for ci in range(3):
                VC = nt_tile()
                tte(VC, c01[ci], G1, ALU.mult)
                tte(TM, c02[ci], G2, ALU.mult)
                tte(VC, VC, TM, ALU.add)
                tte(TM, c12[ci], G3, ALU.mult)
                tte(VC, VC, TM, ALU.add)
                out.append(VC)
            n2v = norm2(out)
            n = nt_tile(sp)
            polished_sqrt(n, n2v, TM)
            nc.vector.tensor_scalar_max(n[:], n[:], 1e-30)
            nc.vector.reciprocal(n[:], n[:])
            for VC in out:
                tte(VC, VC, n, ALU.mult)
            return out

        ZAX = evec(LMIN, nc.gpsimd)
        XAX = evec(LMAX, nc.vector)

        # ---- disambiguation dots (batched over tiles) ----
        def batched_dot(AX, DST, E2=None):
            # DST = sum_c NGH[:, :, c, :] * AX[c] broadcast over K
            E2 = E2 or nc.vector
            tag = "dotg" if E2 is nc.gpsimd else "dottmp"
            tmp = sp.tile([128, NT, K], DT.float32, tag=tag)
            axb = [AX[c][:].rearrange("p t -> p t ()").to_broadcast([128, NT, K])
                   for c in range(3)]
            E2.tensor_tensor(out=DST[:], in0=NGH4[:, :, 0, :], in1=axb[0],
                             op=ALU.mult)
            nc.gpsimd.tensor_tensor(out=tmp[:], in0=NGH4[:, :, 1, :], in1=axb[1],
                                    op=ALU.mult)
            E2.tensor_tensor(out=DST[:], in0=DST[:], in1=tmp[:], op=ALU.add)
            nc.gpsimd.tensor_tensor(out=tmp[:], in0=NGH4[:, :, 2, :], in1=axb[2],
                                    op=ALU.mult)
            E2.tensor_tensor(out=DST[:], in0=DST[:], in1=tmp[:], op=ALU.add)

        DOTX = cp.tile([128, NT, K], DT.float32)
        DOTZ = cp.tile([128, NT, K], DT.float32)
        batched_dot(XAX, DOTX)
        batched_dot(ZAX, DOTZ, E2=nc.gpsimd)

        SGX = cp.tile([128, NT, K], DT.float32)
        SGZ = cp.tile([128, NT, K], DT.float32)
        FX = nt_tile()
        FZ = nt_tile()
        for DOT, F, SG in ((DOTX, FX, SGX), (DOTZ, FZ, SGZ)):
            nc.scalar.activation(SG[:], DOT[:], AF.Sign)
            nc.vector.tensor_reduce(out=F[:], in_=SG[:], axis=mybir.AxisListType.X,
                                    op=ALU.add)
            nc.scalar.activation(F[:], F[:], AF.Sign, bias=0.5, scale=1.0)
        for c in range(3):
            tt(XAX[c], XAX[c], FX, ALU.mult)
            tt(ZAX[c], ZAX[c], FZ, ALU.mult)
        nc.vector.tensor_tensor(
            out=DOTX[:], in0=DOTX[:],
            in1=FX[:].rearrange("p t -> p t ()").to_broadcast([128, NT, K]),
            op=ALU.mult)
        # y = cross(z, x)
        YAX = []
        for (i1, i2) in ((1, 2), (2, 0), (0, 1)):
            YC = nt_tile()
            tt(YC, ZAX[i1], XAX[i2], ALU.mult)
            tt(T1, ZAX[i2], XAX[i1], ALU.mult)
            tt(YC, YC, T1, ALU.subtract)
            YAX.append(YC)
        DOTY = cp.tile([128, NT, K], DT.float32)
        batched_dot(YAX, DOTY)

        # ---- projections (per tile-half) -> PXY (p, t, xy, k) ----
        PXY = cp.tile([128, NT, 2, K], DT.float32)
        PX = PXY[:][:, :, 0, :]
        PY = PXY[:][:, :, 1, :]
        SC = cp.tile([128, NT, K], DT.float32)
        U2 = cp.tile([128, NT, K], DT.float32)
        RCN = cp.tile([128, NT, K], DT.float32)
        S2 = cp.tile([128, NT, K], DT.float32)
        S2T = cp.tile([128, NT, K], DT.float32)
        HT = NT // 2

        def proj_half(th):
            sl = slice(th * HT, (th + 1) * HT)
            dx, dy = DOTX[:, sl, :], DOTY[:, sl, :]
            sc, u2, rcn = SC[:, sl, :], U2[:, sl, :], RCN[:, sl, :]
            px, py = PXY[:, sl, 0, :], PXY[:, sl, 1, :]
            s2, s2t = S2[:, sl, :], S2T[:, sl, :]
            nc.vector.tensor_tensor(out=sc, in0=dx, in1=dx, op=ALU.mult)
            nc.vector.tensor_tensor(out=u2, in0=dy, in1=dy, op=ALU.mult)
            nc.vector.tensor_tensor(out=u2, in0=sc, in1=u2, op=ALU.add)
            nc.scalar.activation(sc, u2, AF.Sqrt)
            # one Newton step: s' = 0.5 (s + u/s) makes sqrt correctly rounded
            nc.vector.tensor_scalar_max(rcn, sc, 1e-30)
            nc.vector.reciprocal(rcn, rcn)
            nc.vector.tensor_tensor(out=rcn, in0=u2, in1=rcn, op=ALU.mult)
            nc.vector.tensor_tensor(out=sc, in0=sc, in1=rcn, op=ALU.add)
            nc.vector.tensor_scalar(out=sc, in0=sc, scalar1=0.5, scalar2=EPS,
                                    op0=ALU.mult, op1=ALU.add)
            nc.vector.reciprocal(sc, sc)
            nc.vector.tensor_tensor(out=sc, in0=sc, in1=DD[:, sl, :], op=ALU.mult)
            nc.vector.tensor_tensor(out=px, in0=dx, in1=sc, op=ALU.mult)
            nc.vector.tensor_tensor(out=py, in0=dy, in1=sc, op=ALU.mult)
            nc.vector.tensor_tensor(out=s2, in0=px, in1=px, op=ALU.mult)
            nc.vector.tensor_tensor(out=s2t, in0=py, in1=py, op=ALU.mult)
            nc.vector.tensor_tensor(out=s2, in0=s2, in1=s2t, op=ALU.add)
            nc.sync.dma_start(
                pxy_o[th * HT * 128:(th + 1) * HT * 128].rearrange(
                    "(t p) x k -> p t x k", p=128),
                PXY[:, sl])

        # ---- BC selection per tile ----
        # Key for cell (i,j), slot k:  d2 = (S2[k] + r_i^2) + r_i * W8[j,k]
        # with W8[j,k] = -2 (cos_j px[k] + sin_j py[k]); then pack slot bits
        # and take the top-3 keys per cell via MAX8. Winner coordinates are
        # gathered on the host from pxy_o (it gathers pidx anyway).
        COSB = CST[:, 0:8].rearrange("p a -> p () a ()").to_broadcast([128, HT, A, K])
        SINB = CST[:, 8:16].rearrange("p a -> p () a ()").to_broadcast([128, HT, A, K])
        RB = CST[:, 16:21].rearrange("p r -> p r () ()").to_broadcast([128, R, A, K])
        R2B = CST[:, 21:26].rearrange("p r -> p () r ()").to_broadcast([128, HT, R, K])
        # W8 and S2+r^2 batched per half so the first tiles start sooner
        W8A = cp.tile([128, NT, A, K], DT.float32)
        T8A = cp.tile([128, NT, A, K], DT.float32)
        S2RA = cp.tile([128, NT, R, K], DT.float32)

        def key_prep(th):
            sl = slice(th * HT, (th + 1) * HT)
            pxab = PX[:, sl, :].rearrange("p t k -> p t () k").to_broadcast(
                [128, HT, A, K])
            pyab = PY[:, sl, :].rearrange("p t k -> p t () k").to_broadcast(
                [128, HT, A, K])
            nc.gpsimd.tensor_tensor(out=T8A[:, sl], in0=pxab, in1=COSB,
                                    op=ALU.mult)
            nc.gpsimd.tensor_tensor(out=W8A[:, sl], in0=pyab, in1=SINB,
                                    op=ALU.mult)
            nc.gpsimd.tensor_tensor(out=W8A[:, sl], in0=W8A[:, sl],
                                    in1=T8A[:, sl], op=ALU.add)
            nc.gpsimd.tensor_tensor(
                out=S2RA[:, sl],
                in0=S2[:, sl, :].rearrange("p t k -> p t () k").to_broadcast(
                    [128, HT, R, K]),
                in1=R2B, op=ALU.add)

        proj_half(0)
        key_prep(0)
        for t in range(NT):
            if t == 2:
                proj_half(1)
            if t == 5:
                key_prep(1)
            RW = bp.tile([128, R, A, K], DT.float32, tag="rw", bufs=3)
            nc.gpsimd.tensor_tensor(
                out=RW[:], in0=RB,
                in1=W8A[:, t].rearrange("p a k -> p () a k").to_broadcast(
                    [128, R, A, K]),
                op=ALU.mult)
            KEY = bp.tile([128, R, A, K], DT.float32, tag="key", bufs=3)
            nc.gpsimd.tensor_tensor(
                out=KEY[:], in0=RW[:],
                in1=S2RA[:, t].rearrange("p r k -> p r () k").to_broadcast(
                    [128, R, A, K]),
                op=ALU.add)
            NKEY = bp.tile([128, NCELL, K], DT.float32, tag="nkey", bufs=3)
            nc.vector.scalar_tensor_tensor(
                out=NKEY[:].bitcast(DT.int32),
                in0=KEY[:].rearrange("p r a k -> p (r a) k").bitcast(DT.int32),
                scalar=M32[:], in1=KIOTA[:], op0=ALU.bitwise_and,
                op1=ALU.bitwise_or)
            M8 = bp.tile([128, NCELL, 8], DT.float32, tag="m8", bufs=3)
            for ra in range(NCELL):
                nc.vector.max(out=M8[:, ra, :], in_=NKEY[:, ra, :])
            M3C = bp.tile([128, NCELL, 3], DT.float32, tag="m3c", bufs=3)
            nc.scalar.copy(M3C[:], M8[:, :, 0:3])
            nc.sync.dma_start(m3_o[t * 128:(t + 1) * 128, :, :], M3C[:])

    split_sync_waits(nc)
    return nc


# ---------------------------------------------------------------------------
# Host glue
# ---------------------------------------------------------------------------


def _split16(x):
    """f32 -> (hi, lo) fp16 pair with hi + lo ~= x."""
    hi = x.astype(np.float16)
    lo = (x - hi.astype(f32)).astype(np.float16)
    return hi, lo


def host_prep_phase1(vertices):
    """vertices (4, 4096, 3) -> list of 8 input maps (13-row fp16 split)."""
    f16 = np.float16
    maps = []
    for core in range(8):
        b, h = core // 2, core % 2
        verts = np.ascontiguousarray(vertices[b], dtype=f32)
        sq = (verts.astype(np.float64) ** 2).sum(-1).astype(f32)
        bh, bl = _split16(verts.T)                     # (3, V) each
        sph, spl = _split16(-sq[None, :])              # (1, V)
        onev = np.ones((1, V), f16)
        pt5 = np.concatenate(
            [bh, bh, bl, bl, sph, spl, onev, onev], axis=0).astype(f16)
        Q = verts[h * HALF:(h + 1) * HALF]
        qsq = sq[h * HALF:(h + 1) * HALF]
        ah, al = _split16(2.0 * Q.T.astype(f32))       # (3, HALF)
        sqh, sql = _split16(-qsq[None, :])
        oneq = np.ones((1, HALF), f16)
        qt5 = np.concatenate(
            [ah, al, ah, al, oneq, oneq, sqh, sql], axis=0).astype(f16)
        maps.append({"pt5": np.ascontiguousarray(pt5),
                     "qt5": np.ascontiguousarray(qt5)})
    return maps


def host_merge(candv, verts, Q):
    """Decode packed candidates, refine exactly, take top-33 by (d2, index).

    candv (HALF, CAND) f32: bits = (-d2 & ~127) | chunk_local_idx, column c
    belongs to chunk c // 8. The device d2 is approximate (fp16-split matmul
    + 7 masked mantissa bits); the top ~48 candidates are re-scored with
    exact f32 distances so the top-33 boundary is noise-free.
    -> nbr (HALF,32) int64, d (HALF,32), radius (HALF,).
    """
    NCAND = 48
    bits = candv.view(np.uint32)
    j = (bits & np.uint32(CHUNKW - 1)).astype(np.int64)
    d2m = -(bits & np.uint32((0xFFFFFFFF << 7) & 0xFFFFFFFF)).view(f32)
    chunk = np.arange(CAND, dtype=np.int64) // 8
    gidx = chunk[None, :] * CHUNKW + j
    part = np.argpartition(d2m, NCAND, axis=1)[:, :NCAND]
    cd = np.take_along_axis(gidx, part, axis=1)              # (HALF, 48)
    diff = verts[cd] - Q[:, None, :]
    d2x = np.einsum("qkc,qkc->qk", diff, diff, dtype=f32).astype(f32)
    order = np.lexsort((cd, d2x), axis=1)[:, :33]
    vals = np.take_along_axis(d2x, order, axis=1)
    idxs = np.take_along_axis(cd, order, axis=1)
    d33 = np.sqrt(np.maximum(vals, 0.0)).astype(f32)
    return idxs[:, :32], d33[:, :32], d33[:, 32]


def host_prep_phase2(vertices, template, p1_results):
    """Build phase-2 input maps + per-core nbr tables from phase-1 outputs."""
    template = np.asarray(template, f32)
    tx = template[..., 0].reshape(-1).astype(f32)
    ty = template[..., 1].reshape(-1).astype(f32)
    # polar factorization of the template grid (it is a polar r x a grid)
    r64 = np.hypot(template[..., 0].astype(np.float64),
                   template[..., 1].astype(np.float64)).mean(axis=1)  # (R,)
    ang = np.arctan2(template[-1, :, 1].astype(np.float64),
                     template[-1, :, 0].astype(np.float64))           # (A,)
    cst_row = np.concatenate([
        -2.0 * np.cos(ang), -2.0 * np.sin(ang), r64, r64 * r64,
        tx.astype(np.float64), ty.astype(np.float64)]).astype(f32)
    cst = np.ascontiguousarray(np.broadcast_to(cst_row[None, :], (128, 106)))
    maps, nbrs = [], []
    for core in range(8):
        b, h = core // 2, core % 2
        verts = np.ascontiguousarray(vertices[b], dtype=f32)
        cv = p1_results[core]["candv"]
        Q = verts[h * HALF:(h + 1) * HALF]
        nbr, d, radius = host_merge(cv, verts, Q)
        neigh = (verts[nbr] - Q[:, None, :]).astype(f32)          # (HALF, 32, 3)
        ngh = np.ascontiguousarray(neigh.transpose(0, 2, 1).reshape(HALF, 96))
        w = (radius[:, None] - d).astype(f32)
        wn = (w / (w.sum(1, keepdims=True, dtype=f32) + f32(EPS))).astype(f32)
        maps.append({"ngh": ngh, "wn": np.ascontiguousarray(wn),
                     "dd": np.ascontiguousarray(d), "cst": cst})
        nbrs.append(nbr)
    return maps, nbrs


def host_assemble(p2_results, nbrs, template):
    """Decode slots, gather winner coords, barycentric weights, assemble output."""
    template = np.asarray(template, f32)
    txy = template.reshape(NCELL, 2)
    out = np.zeros((B, V, R, A, 3, 2), f32)
    one = f32(1.0)
    for core in range(8):
        b, h = core // 2, core % 2
        m3 = np.ascontiguousarray(p2_results[core]["m3o"])        # (HALF, 40, 3)
        pxy = p2_results[core]["pxy"]                             # (HALF, 2, 32)
        k3 = (m3.view(np.int32) & 31).astype(np.int64)            # (HALF, 40, 3)
        nbr = nbrs[core]                                          # (HALF, 32)
        pidx = np.take_along_axis(nbr[:, None, :].repeat(NCELL, 1), k3, axis=2)
        k3f = k3.reshape(HALF, NCELL * 3)
        px = np.take_along_axis(pxy[:, 0, :], k3f, axis=1).reshape(HALF, NCELL, 3)
        py = np.take_along_axis(pxy[:, 1, :], k3f, axis=1).reshape(HALF, NCELL, 3)
        p0x, p1x, p2x = px[..., 0], px[..., 1], px[..., 2]
        p0y, p1y, p2y = py[..., 0], py[..., 1], py[..., 2]
        v0x, v0y = p2x - p0x, p2y - p0y
        v1x, v1y = p1x - p0x, p1y - p0y
        v2x = txy[None, :, 0] - p0x
        v2y = txy[None, :, 1] - p0y
        d00 = v0x * v0x + v0y * v0y
        d01 = v0x * v1x + v0y * v1y
        d02 = v0x * v2x + v0y * v2y
        d11 = v1x * v1x + v1y * v1y
        d12 = v1x * v2x + v1y * v2y
        den = d00 * d11 - d01 * d01 + f32(1e-6)
        w2 = (d11 * d02 - d01 * d12) / den
        w1 = (d00 * d12 - d01 * d02) / den
        w0 = one - w2 - w1
        weights = np.stack([w2, w1, w0], axis=-1).astype(f32)     # (HALF, 40, 3)
        sl = slice(h * HALF, (h + 1) * HALF)
        out[b, sl, ..., 0] = pidx.reshape(HALF, R, A, 3).astype(f32)
        out[b, sl, ..., 1] = weights.reshape(HALF, R, A, 3)
    return out


_PROGS = {}


def _prog(name):
    if name not in _PROGS:
        _PROGS[name] = build_phase1() if name == "p1" else build_phase2()
    return _PROGS[name]


def run_phase1(vertices, trace=False):
    maps = host_prep_phase1(vertices)
    return run_bass_kernel_spmd(_prog("p1"), maps, list(range(8)), trace=trace)


def kernel(vertices, template, trace=False, _timing=None):
    vertices = np.asarray(vertices, f32)
    template = np.asarray(template, f32)

    def _p1_ok(res):
        # packed -d2 must decode to plausible squared distances
        for core in range(8):
            bits = res.results[core]["candv"].view(np.uint32)
            d2m = -(bits & np.uint32(0xFFFFFF80)).view(f32)
            bad = ~((d2m > -1e-3) & (d2m < 4.0))
            if bad.mean() > 1e-4:
                return False
        return True

    def _p2_ok(res, maps2):
        # |p|^2 must match the shipped neighbor distances; slots distinct
        for core in range(8):
            pxy = res.results[core]["pxy"]
            dd = maps2[core]["dd"]
            s2 = pxy[:, 0, :] ** 2 + pxy[:, 1, :] ** 2
            bad = np.abs(s2 - dd * dd) > 1e-2 * (dd * dd + 1e-5)
            if bad.mean() > 1e-3:
                return False
            m3 = np.ascontiguousarray(res.results[core]["m3o"])
            k3 = m3.view(np.int32) & 31
            dup = ((k3[..., 0] == k3[..., 1]) | (k3[..., 1] == k3[..., 2]) |
                   (k3[..., 0] == k3[..., 2]))
            if dup.mean() > 1e-4:
                return False
        return True

    t1_ns = t2_ns = 0
    for _attempt in range(3):
        r1 = run_bass_kernel_spmd(_prog("p1"), host_prep_phase1(vertices),
                                  list(range(8)), trace=trace)
        t1_ns += r1.exec_time_ns or 0
        if _p1_ok(r1):
            break
    maps2, nbrs = host_prep_phase2(vertices, template, r1.results)
    for _attempt in range(3):
        r2 = run_bass_kernel_spmd(_prog("p2"), maps2, list(range(8)), trace=trace)
        t2_ns += r2.exec_time_ns or 0
        if _p2_ok(r2, maps2):
            break
    if _timing is not None:
        _timing["phase1"] = r1
        _timing["phase2"] = r2
        _timing["phase1_ns"] = t1_ns
        _timing["phase2_ns"] = t2_ns
        _timing["maps2"] = maps2
        _timing["nbrs"] = nbrs
    return host_assemble(r2.results, nbrs, template)


if __name__ == "__main__":
    # Phase-1 standalone check against exact numpy KNN.
    cache = np.load("/root/problem/dev_cache/ref.npz")
    vertices = cache["vertices"]
    res = run_phase1(vertices)
    nbad = 0
    for core in range(8):
        b, h = core // 2, core % 2
        verts = vertices[b].astype(f32)
        Q = verts[h * HALF:(h + 1) * HALF]
        d2 = ((Q[:, None, :] - verts[None, :, :]) ** 2).sum(-1)
        ref_order = np.argsort(d2, axis=1, kind="stable")[:, :33]
        nbr, d, rad = host_merge(res.results[core]["candv"], verts, Q)
        rnbr = ref_order[:, :32]
        idx_match = (np.sort(nbr, 1) == np.sort(rnbr, 1)).mean()
        rrad = np.sqrt(np.take_along_axis(d2, ref_order[:, 32:33], axis=1)[:, 0])
        print(f"core {core}: top32 set match={idx_match:.6f} "
              f"rad maxdiff={np.abs(rad-rrad).max():.2e}")
        nbad += (np.sort(nbr, 1) != np.sort(rnbr, 1)).sum()
    print("total nbr mismatches vs exact:", nbad)



# revision 53
# speedup vs baseline: 1.0046x; 1.0046x over previous
"""Barycentric-coordinates KNN kernel for Trainium2 (8 NeuronCores).

Pipeline (per core = one (batch, half-of-V) pair; 8 cores cover 4 batches x 2 halves):
  Phase 1 (device): -d^2 via a single 16-row fp16 split-precision TensorE
    matmul (2q.p - |p|^2 - |q|^2 accumulated in fp32 PSUM, ~1e-6 abs error);
    a 7-bit chunk-local index packed into the mantissa low bits so one DVE
    MAX8 per 128-wide chunk yields fused (value, index) -> 256 candidates/row.
  Host: decode, exact f32 re-score of the top-48 candidates (erases fp16 +
    mask noise), top-33 by (d2, index), neighbor gather, SHOT weights (no
    per-partition gather exists on-chip).
  Phase 2 (device): weighted 3x3 covariance, eigensolver (trig closed-form
    roots of the characteristic cubic via ACT Arctan/Sin + 2 Newton polish
    steps, adjugate cross-products; the two eigenvector chains run
    concurrently on DVE and GpSimd), SHOT sign disambiguation, tangent-plane
    log map -> pxy out; template-cell nearest-3 selection with the polar
    expansion key (S2[k] + r_i^2) - 2 r_i (cos_j px + sin_j py), bit-packed
    (d^2 mantissa | k-slot), MAX8 per cell -> m3 keys out.
  Host: decode k-slots, gather winner coords from pxy, barycentric weights,
    pidx = nbr_idx[closest], assemble (4, 4096, 5, 8, 3, 2) output.
Device outputs are validated (plausible d^2 range, |p|^2 vs shipped
distances, distinct slots) with up to 2 retries to guard against rare
transient hardware flakes.
"""
import sys

sys.path.insert(0, "/opt/trn_rl_repo")

import numpy as np
from contextlib import ExitStack

import concourse.bass as bass
import concourse.mybir as mybir
import concourse.tile as tile
from concourse.bass_utils import run_bass_kernel_spmd
from concourse.tile import ScopedClock

f32 = np.float32
AF = mybir.ActivationFunctionType
ALU = mybir.AluOpType
DT = mybir.dt

B, V, K = 4, 4096, 32
HALF = V // 2            # queries per core
NT = HALF // 128         # 16 v-tiles per core
NCHUNK = 32              # phase-1 chunk count (chunk width 128)
CHUNKW = V // NCHUNK     # 128
CAND = NCHUNK * 8        # 256 candidates per row
R, A = 5, 8
NCELL = R * A            # 40 template cells
EPS = 1e-8
N_RADIAL, N_ANGULAR = 5, 8
TEMPLATE_RADIUS = 0.09

# ---------------------------------------------------------------------------
# Tile-framework workaround: walrus rejects instructions carrying more than a
# couple of sync waits. Spread extras across single-wait NOPs.
# ---------------------------------------------------------------------------


def _patched_drain_and_barrier(self, tick_clock, wait_clock):
    probe = self.nc.sync.nop(nofuse=True)
    wait_clock.add_sem_waits(probe.ins, ScopedClock({None: tick_clock.global_clock}))
    sync_info = probe.ins.sync_info
    waits = list(sync_info.on_wait or []) if sync_info is not None else []
    if len(waits) > 1:
        sync_info.on_wait = waits[:1]
        for i in range(1, len(waits)):
            extra = self.nc.sync.nop(nofuse=True)
            if extra.ins.sync_info is None:
                extra.ins.sync_info = mybir.SyncInfo(on_wait=[waits[i]], on_update=[])
            else:
                extra.ins.sync_info.on_wait = [waits[i]]
    self.nc.sync.drain()
    self.nc.all_engine_barrier()
    assert self.sems is not None
    popped = self.nc._tile_sem_poison_stack.pop()
    assert popped is self._sem_poison
    self.nc.clear_and_free_semaphores(list(self.sems.allocated().values()))
    self.nc.all_engine_barrier()


tile.TileContext._drain_and_barrier = _patched_drain_and_barrier


def split_sync_waits(nc, max_waits=1):
    for f in nc.m.functions:
        for b in f.blocks:
            new_list = []
            dirty = False
            for ins in b.instructions:
                si = ins.sync_info
                waits = list(si.on_wait) if (si is not None and si.on_wait) else []
                if len(waits) > max_waits:
                    dirty = True
                    extras, keep = waits[:-max_waits], waits[-max_waits:]
                    for j in range(0, len(extras), max_waits):
                        nop = mybir.InstNoOp(
                            name=f"I-wsplit-{nc.next_id()}", engine=ins.engine
                        )
                        nop.sync_info = mybir.SyncInfo(
                            on_wait=extras[j : j + max_waits], on_update=[]
                        )
                        new_list.append(nop)
                    si.on_wait = keep
                new_list.append(ins)
            if dirty:
                b.instructions = new_list


# ---------------------------------------------------------------------------
# Phase 1 program
# ---------------------------------------------------------------------------


def build_phase1():
    # -d2(q, p) via one 13-row fp16 split-precision matmul:
    #   2 q.p  = sum_c (ah_c + al_c)(bh_c + bl_c)  (al.bl term dropped)
    #   -|p|^2 = sph + spl,  -|q|^2 = sqh + sql    (hi/lo fp16 splits)
    # accumulated exactly in fp32 PSUM -> -d^2 with ~1e-6 abs error.
    # A 7-bit chunk-local index is packed into the mantissa low bits so a
    # single MAX8 per 128-wide chunk yields (value, index) fused; the host
    # decodes idx = bits & 127.
    nc = bass.Bass()
    NROW = 16
    pt5 = nc.declare_dram_parameter("pt5", [NROW, V], DT.float16, isOutput=False)
    qt5 = nc.declare_dram_parameter("qt5", [NROW, HALF], DT.float16, isOutput=False)
    candv_o = nc.declare_dram_parameter("candv", [HALF, CAND], DT.float32, isOutput=True)

    HC = NCHUNK // 2  # chunks per half (16)

    with tile.TileContext(nc) as tc, ExitStack() as ctx:
        cpool = ctx.enter_context(tc.tile_pool(name="const", bufs=1))
        npool = ctx.enter_context(tc.tile_pool(name="nkey", bufs=3))
        opool = ctx.enter_context(tc.tile_pool(name="cand", bufs=4))
        ppool = ctx.enter_context(tc.tile_pool(name="psum", bufs=2, space="PSUM"))

        pt = cpool.tile([NROW, V], DT.float16)
        qt = cpool.tile([NROW, HALF], DT.float16)
        J7 = cpool.tile([128, 2048], DT.int32)
        M128 = cpool.tile([128, 1], DT.int32)
        nc.sync.dma_start(pt[:], pt5[:])
        nc.sync.dma_start(qt[:], qt5[:])
        nc.gpsimd.iota(J7[:], pattern=[[0, HC], [1, CHUNKW]], base=0,
                       channel_multiplier=0)
        nc.vector.memset(M128[:], -128)

        for t in range(NT):
            for jh in range(2):
                ps = ppool.tile([128, 2048], DT.float32, space="PSUM")
                for k4 in range(4):
                    nc.tensor.matmul(
                        ps[:, k4 * 512:(k4 + 1) * 512],
                        qt[:, t * 128:(t + 1) * 128],
                        pt[:, jh * 2048 + k4 * 512: jh * 2048 + (k4 + 1) * 512],
                        start=True, stop=True,
                    )
                nk = npool.tile([128, 2048], DT.float32, tag="nk")
                nc.vector.scalar_tensor_tensor(
                    out=nk[:].bitcast(DT.int32), in0=ps[:].bitcast(DT.int32),
                    scalar=M128[:], in1=J7[:], op0=ALU.bitwise_and,
                    op1=ALU.bitwise_or)
                cv = opool.tile([128, HC * 8], DT.float32, tag="cv")
                for c in range(HC):
                    nc.vector.max(out=cv[:, c * 8:(c + 1) * 8],
                                  in_=nk[:, c * CHUNKW:(c + 1) * CHUNKW])
                nc.sync.dma_start(
                    candv_o[t * 128:(t + 1) * 128, jh * HC * 8:(jh + 1) * HC * 8],
                    cv[:])

    split_sync_waits(nc)
    return nc


# ---------------------------------------------------------------------------
# Phase 2 program
# ---------------------------------------------------------------------------


def _register_consts(nc, values):
    for value in values:
        t = nc.alloc_sbuf_tensor(f"const-float32-{value}", [128, 1], DT.float32)
        nc.gpsimd.memset(t.ap(), value)
        nc.const_aps.aps[(DT.float32, value)] = t.ap()
    nc.all_engine_barrier()


def build_phase2():
    nc = bass.Bass()
    _register_consts(nc, [0.5, float(np.pi / 2), float(-np.pi / 6)])
    ngh_i = nc.declare_dram_parameter("ngh", [HALF, 96], DT.float32, isOutput=False)
    wn_i = nc.declare_dram_parameter("wn", [HALF, K], DT.float32, isOutput=False)
    dd_i = nc.declare_dram_parameter("dd", [HALF, K], DT.float32, isOutput=False)
    # cst layout: [0:8]=-2cos(a), [8:16]=-2sin(a), [16:21]=r, [21:26]=r^2,
    #             [26:66]=tx, [66:106]=ty   (replicated over partitions)
    cst_i = nc.declare_dram_parameter("cst", [128, 106], DT.float32, isOutput=False)
    m3_o = nc.declare_dram_parameter("m3o", [HALF, NCELL, 3], DT.float32, isOutput=True)
    pxy_o = nc.declare_dram_parameter("pxy", [HALF, 2, K], DT.float32, isOutput=True)

    with tile.TileContext(nc) as tc, ExitStack() as ctx:
        cp = ctx.enter_context(tc.tile_pool(name="const", bufs=1))
        sp = ctx.enter_context(tc.tile_pool(name="scratch", bufs=2))
        bp = ctx.enter_context(tc.tile_pool(name="bc", bufs=2))

        NGH = cp.tile([128, NT, 96], DT.float32)
        WN = cp.tile([128, NT, K], DT.float32)
        DD = cp.tile([128, NT, K], DT.float32)
        CST = cp.tile([128, 106], DT.float32)
        HQ = HALF // 2
        # ngh/wn land first (in halves) so covariance starts before the
        # rest of the input DMA completes
        for th in range(2):
            rs = slice(th * HQ, (th + 1) * HQ)
            ts = slice(th * (NT // 2), (th + 1) * (NT // 2))
            nc.sync.dma_start(NGH[:, ts],
                              ngh_i[rs].rearrange("(t p) c -> p t c", p=128))
            nc.sync.dma_start(WN[:, ts],
                              wn_i[rs].rearrange("(t p) c -> p t c", p=128))
        nc.sync.dma_start(DD[:], dd_i[:].rearrange("(t p) c -> p t c", p=128))
        nc.sync.dma_start(CST[:], cst_i[:])
        TX = CST[:, 26:66]
        TY = CST[:, 66:106]

        KIOTA = cp.tile([128, NCELL, K], DT.int32)
        nc.gpsimd.iota(KIOTA[:], pattern=[[0, NCELL], [1, K]], base=-2147483648,
                       channel_multiplier=0)
        M32 = cp.tile([128, 1], DT.int32)
        nc.vector.memset(M32[:], -32)

        _tagn = [0]

        def nt_tile(pool=cp):
            _tagn[0] += 1
            return pool.tile([128, NT], DT.float32, tag=f"nt{_tagn[0]}",
                             name=f"nt{_tagn[0]}")

        NGH4 = NGH[:].rearrange("p t (c k) -> p t c k", c=3)

        # ---- covariance accumulation (batched per tile-half) ----
        NW = cp.tile([128, NT, 96], DT.float32)
        NW4 = NW[:].rearrange("p t (c k) -> p t c k", c=3)
        CXX, CXY, CXZ, CYY, CYZ, CZZ = [nt_tile() for _ in range(6)]
        cov_dsts = {"xx": CXX, "xy": CXY, "xz": CXZ, "yy": CYY, "yz": CYZ, "zz": CZZ}
        pairs = [("xx", 0, 0), ("xy", 0, 1), ("xz", 0, 2),
                 ("yy", 1, 1), ("yz", 1, 2), ("zz", 2, 2)]
        for th in range(2):
            ts = slice(th * (NT // 2), (th + 1) * (NT // 2))
            HTC = NT // 2
            nc.gpsimd.tensor_tensor(
                out=NW4[:, ts], in0=NGH4[:, ts],
                in1=WN[:, ts].rearrange("p t k -> p t () k").to_broadcast(
                    [128, HTC, 3, K]),
                op=ALU.mult)
            for nmq, a, b in pairs:
                cj = sp.tile([128, HTC, K], DT.float32, tag="covjunk")
                nc.gpsimd.tensor_tensor(out=cj[:], in0=NGH4[:, ts, a, :],
                                        in1=NW4[:, ts, b, :], op=ALU.mult)
                nc.vector.tensor_reduce(out=cov_dsts[nmq][:, ts], in_=cj[:],
                                        axis=mybir.AxisListType.X, op=ALU.add)

        # ---- eigensolver on (128, NT) ----
        def tt(dst, a, bb, op):
            nc.vector.tensor_tensor(out=dst[:], in0=a[:], in1=bb[:], op=op)

        def sq_act(dst, a):
            nc.vector.tensor_tensor(out=dst[:], in0=a[:], in1=a[:], op=ALU.mult)

        Q = nt_tile()
        tt(Q, CXX, CYY, ALU.add)
        tt(Q, Q, CZZ, ALU.add)
        nc.vector.tensor_scalar_mul(Q[:], Q[:], 1.0 / 3.0)
        BXX, BYY, BZZ = nt_tile(), nt_tile(), nt_tile()
        tt(BXX, CXX, Q, ALU.subtract)
        tt(BYY, CYY, Q, ALU.subtract)
        tt(BZZ, CZZ, Q, ALU.subtract)
        P2 = nt_tile()
        T1 = nt_tile(sp)
        sq_act(P2, BXX)
        sq_act(T1, BYY)
        tt(P2, P2, T1, ALU.add)
        sq_act(T1, BZZ)
        tt(P2, P2, T1, ALU.add)
        T2 = nt_tile(sp)
        sq_act(T1, CXY)
        sq_act(T2, CXZ)
        tt(T1, T1, T2, ALU.add)
        sq_act(T2, CYZ)
        tt(T1, T1, T2, ALU.add)
        nc.vector.tensor_scalar_mul(T1[:], T1[:], 2.0)
        tt(P2, P2, T1, ALU.add)
        PP = nt_tile()
        PPX = nt_tile()
        nc.vector.tensor_scalar_mul(PPX[:], P2[:], 1.0 / 6.0)

        def polished_sqrt(dst, x, tmp):
            # ACT Sqrt is ~7e-6; one Newton step s' = (s + x/s)/2 fixes it
            nc.scalar.activation(dst[:], x[:], AF.Sqrt)
            nc.vector.tensor_scalar_max(tmp[:], dst[:], 1e-30)
            nc.vector.reciprocal(tmp[:], tmp[:])
            nc.vector.tensor_tensor(out=tmp[:], in0=x[:], in1=tmp[:], op=ALU.mult)
            nc.vector.tensor_tensor(out=dst[:], in0=dst[:], in1=tmp[:], op=ALU.add)
            nc.vector.tensor_scalar_mul(dst[:], dst[:], 0.5)

        polished_sqrt(PP, PPX, T2)
        PINV = nt_tile()
        nc.vector.tensor_scalar_max(PINV[:], PP[:], 1e-20)
        nc.vector.reciprocal(PINV[:], PINV[:])
        NBXX, NBYY, NBZZ, NBXY, NBXZ, NBYZ = [nt_tile() for _ in range(6)]
        tt(NBXX, BXX, PINV, ALU.mult)
        tt(NBYY, BYY, PINV, ALU.mult)
        tt(NBZZ, BZZ, PINV, ALU.mult)
        tt(NBXY, CXY, PINV, ALU.mult)
        tt(NBXZ, CXZ, PINV, ALU.mult)
        tt(NBYZ, CYZ, PINV, ALU.mult)
        # det(B̂)
        DET = nt_tile()
        sq_act(T1, NBYZ)                     # byz^2
        tt(T2, NBYY, NBZZ, ALU.mult)
        tt(T2, T2, T1, ALU.subtract)
        tt(DET, NBXX, T2, ALU.mult)          # + bxx (byy bzz - byz^2)
        tt(T1, NBXY, NBZZ, ALU.mult)
        tt(T2, NBYZ, NBXZ, ALU.mult)
        tt(T1, T1, T2, ALU.subtract)
        tt(T1, NBXY, T1, ALU.mult)
        tt(DET, DET, T1, ALU.subtract)       # - bxy (bxy bzz - byz bxz)
        tt(T1, NBXY, NBYZ, ALU.mult)
        tt(T2, NBYY, NBXZ, ALU.mult)
        tt(T1, T1, T2, ALU.subtract)
        tt(T1, NBXZ, T1, ALU.mult)
        tt(DET, DET, T1, ALU.add)            # + bxz (bxy byz - byy bxz)
        R2 = nt_tile()                       # 2r = det  clamped to [-2, 2]
        nc.vector.tensor_scalar_min(R2[:], DET[:], 2.0)
        nc.vector.tensor_scalar_max(R2[:], R2[:], -2.0)

        # Roots of beta^3 - 3 beta = 2c via beta = 2 cos(acos(c)/3 - 2 pi k/3):
        # trig seed from ACT Arctan/Sin tables, then 2 Newton polish steps.
        CC = nt_tile()
        nc.vector.tensor_scalar_mul(CC[:], R2[:], 0.5)          # c in [-1, 1]
        OM = nt_tile(sp)
        sq_act(OM, CC)
        nc.vector.tensor_scalar(out=OM[:], in0=OM[:], scalar1=-1.0, scalar2=1.0,
                                op0=ALU.mult, op1=ALU.add)      # 1 - c^2
        nc.vector.tensor_scalar_max(OM[:], OM[:], 1e-12)
        SRT = nt_tile(sp)
        nc.scalar.activation(SRT[:], OM[:], AF.Sqrt)
        nc.vector.tensor_scalar_max(SRT[:], SRT[:], 1e-10)
        nc.vector.reciprocal(SRT[:], SRT[:])
        AT = nt_tile(sp)
        tt(AT, CC, SRT, ALU.mult)                               # tan(arcsin c)
        nc.vector.tensor_scalar_min(AT[:], AT[:], 50.0)
        nc.vector.tensor_scalar_max(AT[:], AT[:], -50.0)
        nc.scalar.activation(AT[:], AT[:], AF.Arctan)           # arcsin(c)
        PHI3 = nt_tile(sp)
        nc.vector.tensor_scalar(out=PHI3[:], in0=AT[:], scalar1=-1.0 / 3.0,
                                scalar2=float(np.pi / 6), op0=ALU.mult,
                                op1=ALU.add)                    # acos(c)/3
        BMAX = nt_tile()
        BMIN = nt_tile()
        # beta_max = 2 cos(phi/3) = 2 sin(pi/2 - phi/3)
        nc.scalar.activation(BMAX[:], PHI3[:], AF.Sin, bias=float(np.pi / 2),
                             scale=-1.0)
        nc.vector.tensor_scalar_mul(BMAX[:], BMAX[:], 2.0)
        # beta_min = 2 cos(phi/3 + 2pi/3) = 2 sin(-pi/6 - phi/3)
        nc.scalar.activation(BMIN[:], PHI3[:], AF.Sin, bias=float(-np.pi / 6),
                             scale=-1.0)
        nc.vector.tensor_scalar_mul(BMIN[:], BMIN[:], 2.0)

        def polish(BETA, E, iters=2):
            FV = nt_tile(sp)
            B2 = nt_tile(sp)
            TP = nt_tile(sp)
            for _ in range(iters):
                E.tensor_tensor(out=B2[:], in0=BETA[:], in1=BETA[:], op=ALU.mult)
                E.tensor_tensor(out=FV[:], in0=B2[:], in1=BETA[:], op=ALU.mult)
                E.tensor_scalar_mul(TP[:], BETA[:], 3.0)
                E.tensor_tensor(out=TP[:], in0=TP[:], in1=FV[:], op=ALU.subtract)
                E.tensor_tensor(out=TP[:], in0=TP[:], in1=R2[:], op=ALU.add)  # -f
                E.tensor_scalar(out=B2[:], in0=B2[:], scalar1=3.0,
                                scalar2=-3.0, op0=ALU.mult, op1=ALU.add)
                E.tensor_scalar_max(B2[:], B2[:], 1e-8)
                nc.vector.reciprocal(B2[:], B2[:])
                E.tensor_tensor(out=TP[:], in0=TP[:], in1=B2[:], op=ALU.mult)
                E.tensor_tensor(out=BETA[:], in0=BETA[:], in1=TP[:], op=ALU.add)

        polish(BMAX, nc.vector)
        polish(BMIN, nc.vector)
        LMAX = nt_tile()
        LMIN = nt_tile()
        tt(LMAX, PP, BMAX, ALU.mult)
        tt(LMAX, LMAX, Q, ALU.add)
        tt(LMIN, PP, BMIN, ALU.mult)
        tt(LMIN, LMIN, Q, ALU.add)

        def evec(lam, E):
            # columns of A - lam I. E = engine for the bulk TT work; is_ge
            # picks and reciprocal stay on DVE (Pool lacks them).
            TM = nt_tile(sp)

            def tte(dst, a, bb, op):
                E.tensor_tensor(out=dst[:], in0=a[:], in1=bb[:], op=op)

            D0, D1, D2 = nt_tile(sp), nt_tile(sp), nt_tile(sp)
            tte(D0, CXX, lam, ALU.subtract)
            tte(D1, CYY, lam, ALU.subtract)
            tte(D2, CZZ, lam, ALU.subtract)
            m0 = (D0, CXY, CXZ)
            m1 = (CXY, D1, CYZ)
            m2 = (CXZ, CYZ, D2)

            def cross(u, v):
                rx, ry, rz = nt_tile(sp), nt_tile(sp), nt_tile(sp)
                tte(rx, u[1], v[2], ALU.mult)
                tte(TM, u[2], v[1], ALU.mult)
                tte(rx, rx, TM, ALU.subtract)
                tte(ry, u[2], v[0], ALU.mult)
                tte(TM, u[0], v[2], ALU.mult)
                tte(ry, ry, TM, ALU.subtract)
                tte(rz, u[0], v[1], ALU.mult)
                tte(TM, u[1], v[0], ALU.mult)
                tte(rz, rz, TM, ALU.subtract)
                return rx, ry, rz

            def norm2(c):
                n = nt_tile(sp)
                tte(n, c[0], c[0], ALU.mult)
                tte(TM, c[1], c[1], ALU.mult)
                tte(n, n, TM, ALU.add)
                tte(TM, c[2], c[2], ALU.mult)
                tte(n, n, TM, ALU.add)
                return n

            c01 = cross(m0, m1)
            c02 = cross(m0, m2)
            c12 = cross(m1, m2)
            n01, n02, n12 = norm2(c01), norm2(c02), norm2(c12)
            G1, G2, G3 = nt_tile(sp), nt_tile(sp), nt_tile(sp)
            tt(G1, n01, n02, ALU.is_ge)
            tt(G2, n01, n12, ALU.is_ge)
            tte(G1, G1, G2, ALU.mult)                   # pick01
            tt(G3, n02, n12, ALU.is_ge)
            U = nt_tile(sp)
            E.tensor_scalar(out=U[:], in0=G1[:], scalar1=-1.0, scalar2=1.0,
                            op0=ALU.mult, op1=ALU.add)   # 1 - pick01
            tte(G2, U, G3, ALU.mult)                    # pick02
            E.tensor_scalar(out=G3[:], in0=G3[:], scalar1=-1.0, scalar2=1.0,
                            op0=ALU.mult, op1=ALU.add)   # 1 - g3
            tte(G3, U, G3, ALU.mult)                    # pick12
            out = []
            for ci in range(3):
                VC = nt_tile()
                tte(VC, c01[ci], G1, ALU.mult)
                tte(TM, c02[ci], G2, ALU.mult)
                tte(VC, VC, TM, ALU.add)
                tte(TM, c12[ci], G3, ALU.mult)
                tte(VC, VC, TM, ALU.add)
                out.append(VC)
            n2v = norm2(out)
            n = nt_tile(sp)
            polished_sqrt(n, n2v, TM)
            nc.vector.tensor_scalar_max(n[:], n[:], 1e-30)
            nc.vector.reciprocal(n[:], n[:])
            for VC in out:
                tte(VC, VC, n, ALU.mult)
            return out

        ZAX = evec(LMIN, nc.gpsimd)
        XAX = evec(LMAX, nc.vector)

        # ---- disambiguation dots (batched over tiles) ----
        def batched_dot(AX, DST, E2=None):
            # DST = sum_c NGH[:, :, c, :] * AX[c] broadcast over K
            E2 = E2 or nc.vector
            tag = "dotg" if E2 is nc.gpsimd else "dottmp"
            tmp = sp.tile([128, NT, K], DT.float32, tag=tag)
            axb = [AX[c][:].rearrange("p t -> p t ()").to_broadcast([128, NT, K])
                   for c in range(3)]
            E2.tensor_tensor(out=DST[:], in0=NGH4[:, :, 0, :], in1=axb[0],
                             op=ALU.mult)
            nc.gpsimd.tensor_tensor(out=tmp[:], in0=NGH4[:, :, 1, :], in1=axb[1],
                                    op=ALU.mult)
            E2.tensor_tensor(out=DST[:], in0=DST[:], in1=tmp[:], op=ALU.add)
            nc.gpsimd.tensor_tensor(out=tmp[:], in0=NGH4[:, :, 2, :], in1=axb[2],
                                    op=ALU.mult)
            E2.tensor_tensor(out=DST[:], in0=DST[:], in1=tmp[:], op=ALU.add)

        DOTX = cp.tile([128, NT, K], DT.float32)
        DOTZ = cp.tile([128, NT, K], DT.float32)
        batched_dot(XAX, DOTX)
        batched_dot(ZAX, DOTZ, E2=nc.gpsimd)

        SGX = cp.tile([128, NT, K], DT.float32)
        SGZ = cp.tile([128, NT, K], DT.float32)
        FX = nt_tile()
        FZ = nt_tile()
        for DOT, F, SG in ((DOTX, FX, SGX), (DOTZ, FZ, SGZ)):
            nc.scalar.activation(SG[:], DOT[:], AF.Sign)
            nc.vector.tensor_reduce(out=F[:], in_=SG[:], axis=mybir.AxisListType.X,
                                    op=ALU.add)
            nc.scalar.activation(F[:], F[:], AF.Sign, bias=0.5, scale=1.0)
        for c in range(3):
            tt(XAX[c], XAX[c], FX, ALU.mult)
            tt(ZAX[c], ZAX[c], FZ, ALU.mult)
        nc.vector.tensor_tensor(
            out=DOTX[:], in0=DOTX[:],
            in1=FX[:].rearrange("p t -> p t ()").to_broadcast([128, NT, K]),
            op=ALU.mult)
        # y = cross(z, x)
        YAX = []
        for (i1, i2) in ((1, 2), (2, 0), (0, 1)):
            YC = nt_tile()
            tt(YC, ZAX[i1], XAX[i2], ALU.mult)
            tt(T1, ZAX[i2], XAX[i1], ALU.mult)
            tt(YC, YC, T1, ALU.subtract)
            YAX.append(YC)
        DOTY = cp.tile([128, NT, K], DT.float32)
        batched_dot(YAX, DOTY)

        # ---- projections (per tile-half) -> PXY (p, t, xy, k) ----
        PXY = cp.tile([128, NT, 2, K], DT.float32)
        PX = PXY[:][:, :, 0, :]
        PY = PXY[:][:, :, 1, :]
        SC = cp.tile([128, NT, K], DT.float32)
        U2 = cp.tile([128, NT, K], DT.float32)
        RCN = cp.tile([128, NT, K], DT.float32)
        S2 = cp.tile([128, NT, K], DT.float32)
        S2T = cp.tile([128, NT, K], DT.float32)
        HT = NT // 2

        def proj_half(th):
            sl = slice(th * HT, (th + 1) * HT)
            dx, dy = DOTX[:, sl, :], DOTY[:, sl, :]
            sc, u2, rcn = SC[:, sl, :], U2[:, sl, :], RCN[:, sl, :]
            px, py = PXY[:, sl, 0, :], PXY[:, sl, 1, :]
            s2, s2t = S2[:, sl, :], S2T[:, sl, :]
            nc.vector.tensor_tensor(out=sc, in0=dx, in1=dx, op=ALU.mult)
            nc.vector.tensor_tensor(out=u2, in0=dy, in1=dy, op=ALU.mult)
            nc.vector.tensor_tensor(out=u2, in0=sc, in1=u2, op=ALU.add)
            nc.scalar.activation(sc, u2, AF.Sqrt)
            # one Newton step: s' = 0.5 (s + u/s) makes sqrt correctly rounded
            nc.vector.tensor_scalar_max(rcn, sc, 1e-30)
            nc.vector.reciprocal(rcn, rcn)
            nc.vector.tensor_tensor(out=rcn, in0=u2, in1=rcn, op=ALU.mult)
            nc.vector.tensor_tensor(out=sc, in0=sc, in1=rcn, op=ALU.add)
            nc.vector.tensor_scalar(out=sc, in0=sc, scalar1=0.5, scalar2=EPS,
                                    op0=ALU.mult, op1=ALU.add)
            nc.vector.reciprocal(sc, sc)
            nc.vector.tensor_tensor(out=sc, in0=sc, in1=DD[:, sl, :], op=ALU.mult)
            nc.vector.tensor_tensor(out=px, in0=dx, in1=sc, op=ALU.mult)
            nc.vector.tensor_tensor(out=py, in0=dy, in1=sc, op=ALU.mult)
            nc.vector.tensor_tensor(out=s2, in0=px, in1=px, op=ALU.mult)
            nc.vector.tensor_tensor(out=s2t, in0=py, in1=py, op=ALU.mult)
            nc.vector.tensor_tensor(out=s2, in0=s2, in1=s2t, op=ALU.add)
            nc.sync.dma_start(
                pxy_o[th * HT * 128:(th + 1) * HT * 128].rearrange(
                    "(t p) x k -> p t x k", p=128),
                PXY[:, sl])

        # ---- BC selection per tile ----
        # Key for cell (i,j), slot k:  d2 = (S2[k] + r_i^2) + r_i * W8[j,k]
        # with W8[j,k] = -2 (cos_j px[k] + sin_j py[k]); then pack slot bits
        # and take the top-3 keys per cell via MAX8. Winner coordinates are
        # gathered on the host from pxy_o (it gathers pidx anyway).
        COSB = CST[:, 0:8].rearrange("p a -> p () a ()").to_broadcast([128, HT, A, K])
        SINB = CST[:, 8:16].rearrange("p a -> p () a ()").to_broadcast([128, HT, A, K])
        RB = CST[:, 16:21].rearrange("p r -> p r () ()").to_broadcast([128, R, A, K])
        R2B = CST[:, 21:26].rearrange("p r -> p () r ()").to_broadcast([128, HT, R, K])
        # W8 and S2+r^2 batched per half so the first tiles start sooner
        W8A = cp.tile([128, NT, A, K], DT.float32)
        T8A = cp.tile([128, NT, A, K], DT.float32)
        S2RA = cp.tile([128, NT, R, K], DT.float32)

        def key_prep(th):
            sl = slice(th * HT, (th + 1) * HT)
            pxab = PX[:, sl, :].rearrange("p t k -> p t () k").to_broadcast(
                [128, HT, A, K])
            pyab = PY[:, sl, :].rearrange("p t k -> p t () k").to_broadcast(
                [128, HT, A, K])
            nc.gpsimd.tensor_tensor(out=T8A[:, sl], in0=pxab, in1=COSB,
                                    op=ALU.mult)
            nc.gpsimd.tensor_tensor(out=W8A[:, sl], in0=pyab, in1=SINB,
                                    op=ALU.mult)
            nc.gpsimd.tensor_tensor(out=W8A[:, sl], in0=W8A[:, sl],
                                    in1=T8A[:, sl], op=ALU.add)
            nc.gpsimd.tensor_tensor(
                out=S2RA[:, sl],
                in0=S2[:, sl, :].rearrange("p t k -> p t () k").to_broadcast(
                    [128, HT, R, K]),
                in1=R2B, op=ALU.add)

        proj_half(0)
        key_prep(0)
        for t in range(NT):
            if t == 2:
                proj_half(1)
            if t == 3:
                key_prep(1)
            RW = bp.tile([128, R, A, K], DT.float32, tag="rw", bufs=3)
            nc.gpsimd.tensor_tensor(
                out=RW[:], in0=RB,
                in1=W8A[:, t].rearrange("p a k -> p () a k").to_broadcast(
                    [128, R, A, K]),
                op=ALU.mult)
            KEY = bp.tile([128, R, A, K], DT.float32, tag="key", bufs=3)
            nc.gpsimd.tensor_tensor(
                out=KEY[:], in0=RW[:],
                in1=S2RA[:, t].rearrange("p r k -> p r () k").to_broadcast(
                    [128, R, A, K]),
                op=ALU.add)
            NKEY = bp.tile([128, NCELL, K], DT.float32, tag="nkey", bufs=3)
            nc.vector.scalar_tensor_tensor(
                out=NKEY[:].bitcast(DT.int32),
                in0=KEY[:].rearrange("p r a k -> p (r a) k").bitcast(DT.int32),
                scalar=M32[:], in1=KIOTA[:], op0=ALU.bitwise_and,
                op1=ALU.bitwise_or)
            M8 = bp.tile([128, NCELL, 8], DT.float32, tag="m8", bufs=3)
            for ra in range(NCELL):
                nc.vector.max(out=M8[:, ra, :], in_=NKEY[:, ra, :])
            M3C = bp.tile([128, NCELL, 3], DT.float32, tag="m3c", bufs=3)
            nc.scalar.copy(M3C[:], M8[:, :, 0:3])
            nc.sync.dma_start(m3_o[t * 128:(t + 1) * 128, :, :], M3C[:])

    split_sync_waits(nc)
    return nc


# ---------------------------------------------------------------------------
# Host glue
# ---------------------------------------------------------------------------


def _split16(x):
    """f32 -> (hi, lo) fp16 pair with hi + lo ~= x."""
    hi = x.astype(np.float16)
    lo = (x - hi.astype(f32)).astype(np.float16)
    return hi, lo


def host_prep_phase1(vertices):
    """vertices (4, 4096, 3) -> list of 8 input maps (13-row fp16 split)."""
    f16 = np.float16
    maps = []
    for core in range(8):
        b, h = core // 2, core % 2
        verts = np.ascontiguousarray(vertices[b], dtype=f32)
        sq = (verts.astype(np.float64) ** 2).sum(-1).astype(f32)
        bh, bl = _split16(verts.T)                     # (3, V) each
        sph, spl = _split16(-sq[None, :])              # (1, V)
        onev = np.ones((1, V), f16)
        pt5 = np.concatenate(
            [bh, bh, bl, bl, sph, spl, onev, onev], axis=0).astype(f16)
        Q = verts[h * HALF:(h + 1) * HALF]
        qsq = sq[h * HALF:(h + 1) * HALF]
        ah, al = _split16(2.0 * Q.T.astype(f32))       # (3, HALF)
        sqh, sql = _split16(-qsq[None, :])
        oneq = np.ones((1, HALF), f16)
        qt5 = np.concatenate(
            [ah, al, ah, al, oneq, oneq, sqh, sql], axis=0).astype(f16)
        maps.append({"pt5": np.ascontiguousarray(pt5),
                     "qt5": np.ascontiguousarray(qt5)})
    return maps


def host_merge(candv, verts, Q):
    """Decode packed candidates, refine exactly, take top-33 by (d2, index).

    candv (HALF, CAND) f32: bits = (-d2 & ~127) | chunk_local_idx, column c
    belongs to chunk c // 8. The device d2 is approximate (fp16-split matmul
    + 7 masked mantissa bits); the top ~48 candidates are re-scored with
    exact f32 distances so the top-33 boundary is noise-free.
    -> nbr (HALF,32) int64, d (HALF,32), radius (HALF,).
    """
    NCAND = 48
    bits = candv.view(np.uint32)
    j = (bits & np.uint32(CHUNKW - 1)).astype(np.int64)
    d2m = -(bits & np.uint32((0xFFFFFFFF << 7) & 0xFFFFFFFF)).view(f32)
    chunk = np.arange(CAND, dtype=np.int64) // 8
    gidx = chunk[None, :] * CHUNKW + j
    part = np.argpartition(d2m, NCAND, axis=1)[:, :NCAND]
    cd = np.take_along_axis(gidx, part, axis=1)              # (HALF, 48)
    diff = verts[cd] - Q[:, None, :]
    d2x = np.einsum("qkc,qkc->qk", diff, diff, dtype=f32).astype(f32)
    order = np.lexsort((cd, d2x), axis=1)[:, :33]
    vals = np.take_along_axis(d2x, order, axis=1)
    idxs = np.take_along_axis(cd, order, axis=1)
    d33 = np.sqrt(np.maximum(vals, 0.0)).astype(f32)
    return idxs[:, :32], d33[:, :32], d33[:, 32]


def host_prep_phase2(vertices, template, p1_results):
    """Build phase-2 input maps + per-core nbr tables from phase-1 outputs."""
    template = np.asarray(template, f32)
    tx = template[..., 0].reshape(-1).astype(f32)
    ty = template[..., 1].reshape(-1).astype(f32)
    # polar factorization of the template grid (it is a polar r x a grid)
    r64 = np.hypot(template[..., 0].astype(np.float64),
                   template[..., 1].astype(np.float64)).mean(axis=1)  # (R,)
    ang = np.arctan2(template[-1, :, 1].astype(np.float64),
                     template[-1, :, 0].astype(np.float64))           # (A,)
    cst_row = np.concatenate([
        -2.0 * np.cos(ang), -2.0 * np.sin(ang), r64, r64 * r64,
        tx.astype(np.float64), ty.astype(np.float64)]).astype(f32)
    cst = np.ascontiguousarray(np.broadcast_to(cst_row[None, :], (128, 106)))
    maps, nbrs = [], []
    for core in range(8):
        b, h = core // 2, core % 2
        verts = np.ascontiguousarray(vertices[b], dtype=f32)
        cv = p1_results[core]["candv"]
        Q = verts[h * HALF:(h + 1) * HALF]
        nbr, d, radius = host_merge(cv, verts, Q)
        neigh = (verts[nbr] - Q[:, None, :]).astype(f32)          # (HALF, 32, 3)
        ngh = np.ascontiguousarray(neigh.transpose(0, 2, 1).reshape(HALF, 96))
        w = (radius[:, None] - d).astype(f32)
        wn = (w / (w.sum(1, keepdims=True, dtype=f32) + f32(EPS))).astype(f32)
        maps.append({"ngh": ngh, "wn": np.ascontiguousarray(wn),
                     "dd": np.ascontiguousarray(d), "cst": cst})
        nbrs.append(nbr)
    return maps, nbrs


def host_assemble(p2_results, nbrs, template):
    """Decode slots, gather winner coords, barycentric weights, assemble output."""
    template = np.asarray(template, f32)
    txy = template.reshape(NCELL, 2)
    out = np.zeros((B, V, R, A, 3, 2), f32)
    one = f32(1.0)
    for core in range(8):
        b, h = core // 2, core % 2
        m3 = np.ascontiguousarray(p2_results[core]["m3o"])        # (HALF, 40, 3)
        pxy = p2_results[core]["pxy"]                             # (HALF, 2, 32)
        k3 = (m3.view(np.int32) & 31).astype(np.int64)            # (HALF, 40, 3)
        nbr = nbrs[core]                                          # (HALF, 32)
        pidx = np.take_along_axis(nbr[:, None, :].repeat(NCELL, 1), k3, axis=2)
        k3f = k3.reshape(HALF, NCELL * 3)
        px = np.take_along_axis(pxy[:, 0, :], k3f, axis=1).reshape(HALF, NCELL, 3)
        py = np.take_along_axis(pxy[:, 1, :], k3f, axis=1).reshape(HALF, NCELL, 3)
        p0x, p1x, p2x = px[..., 0], px[..., 1], px[..., 2]
        p0y, p1y, p2y = py[..., 0], py[..., 1], py[..., 2]
        v0x, v0y = p2x - p0x, p2y - p0y
        v1x, v1y = p1x - p0x, p1y - p0y
        v2x = txy[None, :, 0] - p0x
        v2y = txy[None, :, 1] - p0y
        d00 = v0x * v0x + v0y * v0y
        d01 = v0x * v1x + v0y * v1y
        d02 = v0x * v2x + v0y * v2y
        d11 = v1x * v1x + v1y * v1y
        d12 = v1x * v2x + v1y * v2y
        den = d00 * d11 - d01 * d01 + f32(1e-6)
        w2 = (d11 * d02 - d01 * d12) / den
        w1 = (d00 * d12 - d01 * d02) / den
        w0 = one - w2 - w1
        weights = np.stack([w2, w1, w0], axis=-1).astype(f32)     # (HALF, 40, 3)
        sl = slice(h * HALF, (h + 1) * HALF)
        out[b, sl, ..., 0] = pidx.reshape(HALF, R, A, 3).astype(f32)
        out[b, sl, ..., 1] = weights.reshape(HALF, R, A, 3)
    return out


_PROGS = {}


def _prog(name):
    if name not in _PROGS:
        _PROGS[name] = build_phase1() if name == "p1" else build_phase2()
    return _PROGS[name]


def run_phase1(vertices, trace=False):
    maps = host_prep_phase1(vertices)
    return run_bass_kernel_spmd(_prog("p1"), maps, list(range(8)), trace=trace)


def kernel(vertices, template, trace=False, _timing=None):
    vertices = np.asarray(vertices, f32)
    template = np.asarray(template, f32)

    def _p1_ok(res):
        # packed -d2 must decode to plausible squared distances
        for core in range(8):
            bits = res.results[core]["candv"].view(np.uint32)
            d2m = -(bits & np.uint32(0xFFFFFF80)).view(f32)
            bad = ~((d2m > -1e-3) & (d2m < 4.0))
            if bad.mean() > 1e-4:
                return False
        return True

    def _p2_ok(res, maps2):
        # |p|^2 must match the shipped neighbor distances; slots distinct
        for core in range(8):
            pxy = res.results[core]["pxy"]
            dd = maps2[core]["dd"]
            s2 = pxy[:, 0, :] ** 2 + pxy[:, 1, :] ** 2
            bad = np.abs(s2 - dd * dd) > 1e-2 * (dd * dd + 1e-5)
            if bad.mean() > 1e-3:
                return False
            m3 = np.ascontiguousarray(res.results[core]["m3o"])
            k3 = m3.view(np.int32) & 31
            dup = ((k3[..., 0] == k3[..., 1]) | (k3[..., 1] == k3[..., 2]) |
                   (k3[..., 0] == k3[..., 2]))
            if dup.mean() > 1e-4:
                return False
        return True

    t1_ns = t2_ns = 0
    for _attempt in range(3):
        r1 = run_bass_kernel_spmd(_prog("p1"), host_prep_phase1(vertices),
                                  list(range(8)), trace=trace)
        t1_ns += r1.exec_time_ns or 0
        if _p1_ok(r1):
            break
    maps2, nbrs = host_prep_phase2(vertices, template, r1.results)
    for _attempt in range(3):
        r2 = run_bass_kernel_spmd(_prog("p2"), maps2, list(range(8)), trace=trace)
        t2_ns += r2.exec_time_ns or 0
        if _p2_ok(r2, maps2):
            break
    if _timing is not None:
        _timing["phase1"] = r1
        _timing["phase2"] = r2
        _timing["phase1_ns"] = t1_ns
        _timing["phase2_ns"] = t2_ns
        _timing["maps2"] = maps2
        _timing["nbrs"] = nbrs
    return host_assemble(r2.results, nbrs, template)


if __name__ == "__main__":
    # Phase-1 standalone check against exact numpy KNN.
    cache = np.load("/root/problem/dev_cache/ref.npz")
    vertices = cache["vertices"]
    res = run_phase1(vertices)
    nbad = 0
    for core in range(8):
        b, h = core // 2, core % 2
        verts = vertices[b].astype(f32)
        Q = verts[h * HALF:(h + 1) * HALF]
        d2 = ((Q[:, None, :] - verts[None, :, :]) ** 2).sum(-1)
        ref_order = np.argsort(d2, axis=1, kind="stable")[:, :33]
        nbr, d, rad = host_merge(res.results[core]["candv"], verts, Q)
        rnbr = ref_order[:, :32]
        idx_match = (np.sort(nbr, 1) == np.sort(rnbr, 1)).mean()
        rrad = np.sqrt(np.take_along_axis(d2, ref_order[:, 32:33], axis=1)[:, 0])
        print(f"core {core}: top32 set match={idx_match:.6f} "
              f"rad maxdiff={np.abs(rad-rrad).max():.2e}")
        nbad += (np.sort(nbr, 1) != np.sort(rnbr, 1)).sum()
    print("total nbr mismatches vs exact:", nbad)

